# revision 24
# baseline (speedup 1.0000x reference)
"""Trainium2 Bass kernel for T5-style cross-attention, sharded over 8 NeuronCores.

Sharding: tensor-parallel over heads (16 heads -> 2 per core). Each core
computes Q/K/V projections for its 2 heads (full batch), flash-style
attention with multiplicative exp(position_bias), and a partial output
projection against its row-slice of Wo. The host sums the 8 fp16 partial
outputs (the unshard step for a row-sharded Wo).

v8: cost-model-driven restructure of v7 (engine-occupancy analysis showed
ACT saturated by exp with PE stalling on PSUM-ring rotation, a 21us
DMA-issue-bound start, and a 10us tail):
- Weights pre-transposed on host -> 1 DMA each (was 26 small DMAs
  serializing the start on the sync queue).
- exp(bias) packed into quad tiles on host -> 16 SWDGE DMAs (was 64),
  ~1us of Pool descriptor-gen each.
- DMA queue ownership: SP = x loads + weights + out stores (out stores
  only run in windows where SP has no loads left -> no head-of-line
  blocking); Pool(SWDGE) = enc + ebias (engine-time descgen, but Pool has
  slack); no DMAs on ACT/DVE queues.
- PSUM split into three independent rings: scores (2 x [128,1024]f32),
  u (ctx accumulator), small (2 x [128,512] slots shared by projection
  chunks, V-transpose staging, and Wo half-units) so score-tile rotation
  only couples to exp, not to Wo/proj evacuation.
- Projections emit [128,512] psum chunks; Q/K chunk evacuation on ACT,
  V on DVE; ACT otherwise runs only exp (the binding engine).
- Wo units emit two [128,512] halves; evacuation split DVE 2/3, ACT 1/3.
- Last window's Wo units emit inline after each batch's normalization
  (removes the serialized drain tail).
"""

import sys

try:
    import concourse.bass as bass
except ImportError:
    sys.path.insert(0, "/opt/trn_rl_repo")
    import concourse.bass as bass

import numpy as np
import ml_dtypes
_bf16 = ml_dtypes.bfloat16

import concourse.mybir as mybir
from concourse import bacc
from concourse.tile import TileContext
from concourse.bass_utils import run_bass_kernel_spmd

F32 = mybir.dt.float32
F16 = mybir.dt.float16
BF16 = mybir.dt.bfloat16

# Problem sizes (hardcoded per spec)
B, NQ, NKV = 4, 2048, 2048
D_MODEL, N_HEADS, D_K = 1024, 16, 64
N_CORES = 8
HPC = N_HEADS // N_CORES          # heads per core = 2
DH = HPC * D_K                    # 128 partition rows of per-core head dims

QW = 512                          # flash q window
KT = 128                          # k tile (partition dim of S^T)
KQUAD = 4                         # ebias kt tiles packed per DMA
LAG = 5                           # ctx matmul software-pipeline depth


def build_kernel(b=B, nq=NQ, nkv=NKV, d_model=D_MODEL, reps=1,
                 variant="base"):
    nc = bacc.Bacc("TRN2", target_bir_lowering=False, debug=False,
                   num_devices=N_CORES)

    n_m = d_model // 128          # model-dim tiles (8)
    n_qw = nq // QW               # flash q windows (4)
    n_kt = nkv // KT              # k tiles (16)
    n_quad = n_kt // KQUAD        # ebias quads (4)

    xT = nc.dram_tensor("xT", [b, d_model, nq], F16, kind="ExternalInput")
    encT = nc.dram_tensor("encT", [b, d_model, nkv], F16,
                          kind="ExternalInput")
    # exp(position_bias), host-packed quads: [qw, quad, p, kt4*1024 + h*QW+q]
    ebias = nc.dram_tensor("ebias", [n_qw, n_quad, KT, KQUAD * HPC * QW],
                           BF16, kind="ExternalInput")
    # host-pretransposed: [p, m*DH+j] = W[m*128+p, j]
    wq = nc.dram_tensor("wq", [128, n_m * DH], F16, kind="ExternalInput")
    wk = nc.dram_tensor("wk", [128, n_m * DH], F16, kind="ExternalInput")
    wv = nc.dram_tensor("wv", [128, n_m * DH], F16, kind="ExternalInput")
    wo = nc.dram_tensor("wo", [DH, d_model], F16, kind="ExternalInput")
    ident16 = nc.dram_tensor("ident16", [128, 128], F16,
                             kind="ExternalInput")
    out = nc.dram_tensor("out", [b, nq, d_model], F16, kind="ExternalOutput")

    with TileContext(nc) as tc:
        with (
            tc.tile_pool(name="cst", bufs=1) as cst,
            tc.tile_pool(name="wpool", bufs=1) as wpool,
            tc.tile_pool(name="qkv", bufs=1) as qkv,
            tc.tile_pool(name="stage", bufs=12) as stage,
            tc.tile_pool(name="sbias", bufs=5) as sbias,
            tc.tile_pool(name="sattn", bufs=4 * LAG) as sattn,
            tc.tile_pool(name="sctx", bufs=2 * b) as sctx,
            tc.tile_pool(name="vtstage", bufs=2) as vtstage,
            tc.tile_pool(name="sout", bufs=3) as sout,
            tc.tile_pool(name="ssmall", bufs=4) as ssmall,
            tc.tile_pool(name="psA", bufs=2, space="PSUM") as psA,
            tc.tile_pool(name="ps_u", bufs=1, space="PSUM") as ps_u,
            tc.tile_pool(name="psB", bufs=2, space="PSUM") as psB,
        ):
            # ---- constants & weights (one DMA each, SP queue) ----
            ident = cst.tile([128, 128], F16, tag="ident")
            nc.sync.dma_start(out=ident, in_=ident16[:, :])

            wq_sb = wpool.tile([128, n_m * DH], F16, tag="wq")
            wk_sb = wpool.tile([128, n_m * DH], F16, tag="wk")
            wv_sb = wpool.tile([128, n_m * DH], F16, tag="wv")
            nc.sync.dma_start(out=wq_sb, in_=wq[:, :])
            nc.sync.dma_start(out=wk_sb, in_=wk[:, :])
            nc.sync.dma_start(out=wv_sb, in_=wv[:, :])
            wo_sb = wpool.tile([128, d_model], F16, tag="wo")
            nc.sync.dma_start(out=wo_sb, in_=wo[:, :])

            qT_sb = qkv.tile([128, b * nq], F16, tag="qT")
            kT_sb = qkv.tile([128, b * nkv], F16, tag="kT")
            # pair-packed Vones tiles: [h0 V(64) | ones | h1 V(64) | ones]
            vones = {}
            for bi in range(b):
                for kt in range(n_kt):
                    vones[(bi, kt)] = qkv.tile(
                        [128, 2 * (D_K + 1)], BF16, tag=f"v_{bi}_{kt}",
                        name=f"v_{bi}_{kt}")

            lp = nc.allow_low_precision(reason="fp16/bf16 attention pipeline")
            lp.__enter__()
            for rep in range(reps):
                emit_body(nc, tc, rep, b, nq, nkv, d_model, n_m, n_qw, n_kt,
                          n_quad, stage, sbias, sattn, sctx, vtstage, sout,
                          ssmall, psA, ps_u, psB, qT_sb, kT_sb, vones, wq_sb,
                          wk_sb, wv_sb, wo_sb, ident, xT, encT, ebias, out,
                          variant)
            lp.__exit__(None, None, None)
    nc.compile()
    return nc


def emit_body(nc, tc, rep, b, nq, nkv, d_model, n_m, n_qw, n_kt, n_quad,
              stage, sbias, sattn, sctx, vtstage, sout, ssmall,
              psA, ps_u, psB, qT_sb, kT_sb, vones, wq_sb, wk_sb,
              wv_sb, wo_sb, ident, xT, encT, ebias, out, variant="base"):
            import concourse.mybir as mybir

            # ---- projections for one batch, as a list of pieces ----
            # Each piece is one [128,512] psum chunk (+ its input DMAs /
            # evacuation / V transposes). Pieces of batch bi+1 are emitted
            # interleaved into bi's attention pair-loop so the in-order PE
            # queue can fill exp-round-trip stalls with projection matmuls.
            def proj_pieces(bi):
                pieces = []
                xt = {}
                et = {}

                def load_x(pw):
                    for m in range(n_m):
                        xt[(pw, m)] = stage.tile(
                            [128, 1024], F16, tag="stage",
                            name=f"x_{rep}_{bi}_{pw}_{m}")
                        nc.sync.dma_start(
                            out=xt[(pw, m)],
                            in_=xT[bi, m * 128:(m + 1) * 128,
                                   pw * 1024:(pw + 1) * 1024])

                def load_e(pw):
                    for m in range(n_m):
                        et[(pw, m)] = stage.tile(
                            [128, 1024], F16, tag="stage",
                            name=f"e_{rep}_{bi}_{pw}_{m}")
                        nc.gpsimd.dma_start(
                            out=et[(pw, m)],
                            in_=encT[bi, m * 128:(m + 1) * 128,
                                     pw * 1024:(pw + 1) * 1024])

                def q_chunk(pw, s):
                    c = pw * 2 + s
                    q_ps = psB.tile([128, 512], F32, tag="b",
                                    name=f"qps_{rep}_{bi}_{c}")
                    for m in range(n_m):
                        nc.tensor.matmul(
                            q_ps,
                            wq_sb[:, m * DH:(m + 1) * DH],
                            xt[(pw, m)][:, s * 512:(s + 1) * 512],
                            start=(m == 0), stop=(m == n_m - 1))
                    nc.vector.tensor_copy(
                        qT_sb[:, bi * nq + c * 512:bi * nq + (c + 1) * 512],
                        q_ps)

                def k_chunk(pw, s):
                    c = pw * 2 + s
                    k_ps = psB.tile([128, 512], F32, tag="b",
                                    name=f"kps_{rep}_{bi}_{c}")
                    for m in range(n_m):
                        nc.tensor.matmul(
                            k_ps,
                            wk_sb[:, m * DH:(m + 1) * DH],
                            et[(pw, m)][:, s * 512:(s + 1) * 512],
                            start=(m == 0), stop=(m == n_m - 1))
                    nc.vector.tensor_copy(
                        kT_sb[:, bi * nkv + c * 512:
                              bi * nkv + (c + 1) * 512],
                        k_ps)

                def v_chunk(pw, s):
                    c = pw * 2 + s
                    v_ps = psB.tile([128, 512], F32, tag="b",
                                    name=f"vps_{rep}_{bi}_{c}")
                    for m in range(n_m):
                        nc.tensor.matmul(
                            v_ps,
                            wv_sb[:, m * DH:(m + 1) * DH],
                            et[(pw, m)][:, s * 512:(s + 1) * 512],
                            start=(m == 0), stop=(m == n_m - 1))
                    vt_win = vtstage.tile([128, 512], F16, tag="vtw")
                    nc.vector.tensor_copy(vt_win, v_ps)
                    vtbig = psB.tile([128, 512], F16, tag="b",
                                     name=f"vtb_{rep}_{bi}_{c}")
                    for j in range(512 // KT):
                        kt = c * (512 // KT) + j
                        nc.tensor.transpose(
                            vtbig[:, j * KT:(j + 1) * KT],
                            vt_win[:, j * KT:(j + 1) * KT], ident)
                        vt = vones[(bi, kt)]
                        for h in range(HPC):
                            o = h * (D_K + 1)
                            nc.vector.tensor_copy(
                                vt[:, o:o + D_K],
                                vtbig[:, j * KT + h * D_K:
                                      j * KT + (h + 1) * D_K])
                            nc.vector.memset(
                                vt[:, o + D_K:o + D_K + 1], 1.0)

                def piece(fn, *a, pre=None):
                    def run():
                        if pre is not None:
                            pre()
                        fn(*a)
                    return run

                for pw in range(nq // 1024):
                    pieces.append(piece(q_chunk, pw, 0,
                                        pre=(lambda p=pw: load_x(p))))
                    pieces.append(piece(q_chunk, pw, 1))
                for pw in range(nkv // 1024):
                    pieces.append(piece(k_chunk, pw, 0,
                                        pre=(lambda p=pw: load_e(p))))
                    pieces.append(piece(k_chunk, pw, 1))
                    pieces.append(piece(v_chunk, pw, 0))
                    pieces.append(piece(v_chunk, pw, 1))
                return pieces

            # ---- output projection, one 128-row unit at a time ----
            wo_unit_idx = [0]

            def emit_wo_unit(pq0, pctx, ubi, qs):
                o_sb = sout.tile([128, d_model], F16, tag="out")
                for e in range(d_model // 512):
                    o_ps = psB.tile([128, 512], F32, tag="b",
                                    name=f"ops_{rep}_{pq0}_{ubi}_{qs}_{e}")
                    nc.tensor.matmul(
                        o_ps,
                        pctx[ubi][:, qs * 128:(qs + 1) * 128],
                        wo_sb[:, e * 512:(e + 1) * 512],
                        start=True, stop=True)
                    nc.vector.tensor_copy(
                        o_sb[:, e * 512:(e + 1) * 512], o_ps)
                wo_unit_idx[0] += 1
                nc.sync.dma_start(
                    out=out[ubi, pq0 + qs * 128:pq0 + (qs + 1) * 128, :],
                    in_=o_sb)

            # ---- phase B: flash attention, software-pipelined ----
            if True:
                EBPF = 3              # ebias quad prefetch distance
                pending_units = []
                norm_q = []

                def emit_norm():
                    nbi, nu, nctx_t, nq0, nlast = norm_q.pop(0)
                    for h in range(HPC):
                        hp = h * D_K
                        usrc = nu[:, h * QW:(h + 1) * QW]
                        recip = ssmall.tile([1, QW], F32, tag="recip",
                                            name=f"recip_{rep}_{h}")
                        nc.vector.reciprocal(recip, usrc[D_K:D_K + 1, :])
                        rb = ssmall.tile([D_K, QW], F32, tag="rb",
                                         name=f"rb_{rep}_{h}")
                        nc.gpsimd.partition_broadcast(rb, recip)
                        nc.vector.tensor_mul(
                            nctx_t[nbi][hp:hp + D_K, :],
                            usrc[0:D_K, :], rb)
                    if nlast:
                        # final window: drain this batch's units now
                        for qs in range(QW // 128):
                            emit_wo_unit(nq0, nctx_t, nbi, qs)

                def load_eb(eb_sb, qw, quad):
                    eb_sb[quad] = sbias.tile(
                        [128, KQUAD * HPC * QW], BF16, tag="bias",
                        name=f"eb_{rep}_{qw}_{quad}")
                    nc.gpsimd.dma_start(out=eb_sb[quad], in_=ebias[qw, quad])

                for qw in range(n_qw):
                    q0 = qw * QW
                    last_w = qw == n_qw - 1
                    # prefetch the first few exp(bias) quads; the rest are
                    # issued inside bi==0's k loop. At qw==0 the prefetch
                    # is deferred until after bi0's projection DMAs so the
                    # ebias quads don't crowd the x slabs off the DMA
                    # engines at kernel start.
                    eb_sb = {}
                    if qw > 0:
                        for quad in range(min(EBPF, n_quad)):
                            load_eb(eb_sb, qw, quad)
                    ctx_t = [sctx.tile([128, QW], F16, tag="ctx",
                                       name=f"ctx_{rep}_{qw}_{bi}")
                             for bi in range(b)]
                    for bi in range(b):
                        if qw == 0:
                            if bi == 0:
                                for p in proj_pieces(0):
                                    p()
                                for quad in range(min(EBPF, n_quad)):
                                    load_eb(eb_sb, qw, quad)
                            next_pieces = (proj_pieces(bi + 1)
                                           if bi + 1 < b else [])
                        else:
                            next_pieces = []
                        u = ps_u.tile([D_K + 1, 2 * QW], F32, tag="u",
                                      name=f"u_{rep}_{qw}_{bi}")
                        pend = []

                        def issue_ctx(item):
                            kt_i, attnb_i = item
                            for h in range(HPC):
                                o = h * (D_K + 1)
                                # kt==0 starts the bank (clears has_written
                                # + overwrites): no pre-zero memset needed
                                nc.tensor.matmul(
                                    u[:, h * QW:(h + 1) * QW],
                                    vones[(bi, kt_i)][:, o:o + D_K + 1],
                                    attnb_i[:, h * QW:(h + 1) * QW],
                                    start=(kt_i == 0),
                                    stop=(kt_i == n_kt - 1),
                                    skip_group_check=True)

                        # kt processed in PAIRS: the two groups' score
                        # matmuls (64x128 row-tiled mode) are batched, and
                        # the pipelined ctx matmuls + Wo units (128x128
                        # mode) are batched adjacently -- halves the PE
                        # tiling-mode switches, each of which drains the
                        # array on hardware.
                        def emit_scores(kt):
                            s_g = psA.tile([128, 2 * QW], F32, tag="score",
                                           name="sg")
                            for h in range(HPC):
                                hp = 0 if variant == "seqscores" else h * D_K
                                nc.tensor.matmul(
                                    s_g[:, h * QW:(h + 1) * QW],
                                    kT_sb[hp:hp + D_K,
                                          bi * nkv + kt * KT:
                                          bi * nkv + (kt + 1) * KT],
                                    qT_sb[hp:hp + D_K,
                                          bi * nq + q0:bi * nq + q0 + QW],
                                    start=True, stop=True)
                            return s_g

                        def emit_expmul(kt, s_g):
                            attn = sattn.tile([128, 2 * QW], BF16,
                                              tag="attn", name="at")
                            nc.scalar.activation(
                                attn, s_g, mybir.ActivationFunctionType.Exp)
                            if variant == "nomul":
                                attnb = attn
                            else:
                                attnb = sattn.tile([128, 2 * QW], BF16,
                                                   tag="attn", name="ab")
                                quad, ko = kt // KQUAD, kt % KQUAD
                                nc.vector.tensor_mul(
                                    attnb, attn,
                                    eb_sb[quad][:, ko * 1024:(ko + 1) * 1024])
                            pend.append((kt, attnb))

                        for kp in range(n_kt // 2):
                            kt0, kt1 = 2 * kp, 2 * kp + 1
                            if (bi == 0 and kt0 % KQUAD == 0
                                    and kt0 // KQUAD + EBPF < n_quad):
                                load_eb(eb_sb, qw, kt0 // KQUAD + EBPF)
                            if norm_q and kt0 == 2:
                                emit_norm()
                            # 128x128-mode batch: Wo unit + pipelined ctx
                            gidx = bi * n_kt + kt0
                            if pending_units and gidx % 4 == 0:
                                emit_wo_unit(*pending_units.pop(0))
                            while len(pend) > LAG:
                                issue_ctx(pend.pop(0))
                            # qw0: next batch's projection chunks fill the
                            # 128x128-mode batch region
                            for _ in range(2):
                                if next_pieces:
                                    next_pieces.pop(0)()
                            # 64x128-mode batch: both groups' scores
                            sg0 = emit_scores(kt0)
                            sg1 = emit_scores(kt1)
                            emit_expmul(kt0, sg0)
                            emit_expmul(kt1, sg1)
                        for item in pend:
                            issue_ctx(item)
                        # normalization is deferred into the next batch's
                        # kt loop (see emit_norm hook at kt==2): emitting
                        # it here would head-of-line block the Pool queue
                        # (partition_broadcast waits on recip and the next
                        # batch's enc DMA descgen sits behind it in qw0)
                        norm_q.append((bi, u, ctx_t, q0, last_w))
                    if not last_w:
                        pending_units = [
                            (q0, ctx_t, ubi, qs)
                            for ubi in range(b) for qs in range(QW // 128)]
                while norm_q:
                    emit_norm()
                for unit in pending_units:
                    emit_wo_unit(*unit)


_NC_CACHE = {}


def _get_nc():
    if "nc" not in _NC_CACHE:
        _NC_CACHE["nc"] = build_kernel()
    return _NC_CACHE["nc"]


def _prep_inputs(x, encoding, position_bias, Wq, Wk, Wv, Wo):
    x = np.asarray(x, np.float32)
    encoding = np.asarray(encoding, np.float32)
    position_bias = np.asarray(position_bias, np.float32)

    xT = np.ascontiguousarray(
        x.transpose(0, 2, 1)).astype(np.float16)
    encT = np.ascontiguousarray(
        encoding.transpose(0, 2, 1)).astype(np.float16)
    ident16 = np.eye(128, dtype=np.float16)

    n_qw = NQ // QW
    n_kt = NKV // KT
    n_quad = n_kt // KQUAD
    n_m = D_MODEL // 128

    def wslice_t(W, h0):
        # [d_model, DH] -> [128, n_m*DH] with [p, m*DH+j] = W[m*128+p, j]
        Ws = np.ascontiguousarray(W[:, h0 * D_K:(h0 + HPC) * D_K])
        return np.ascontiguousarray(
            Ws.reshape(n_m, 128, DH).transpose(1, 0, 2).reshape(
                128, n_m * DH)).astype(np.float16)

    in_maps = []
    for c in range(N_CORES):
        h0 = c * HPC
        # exp(bias) pre-tiled quads: [qw, quad, p, kq*(HPC*QW) + h*QW + q]
        eb = np.exp(position_bias[0, h0:h0 + HPC])           # [h, q, k]
        eb = eb.reshape(HPC, n_qw, QW, n_kt, KT)             # h,qw,q,kt,p
        eb = np.ascontiguousarray(eb.transpose(1, 3, 4, 0, 2)).reshape(
            n_qw, n_kt, KT, HPC * QW)                        # qw,kt,p,hq
        eb = np.ascontiguousarray(
            eb.reshape(n_qw, n_quad, KQUAD, KT, HPC * QW).transpose(
                0, 1, 3, 2, 4)).reshape(
            n_qw, n_quad, KT, KQUAD * HPC * QW).astype(_bf16)
        in_maps.append({
            "xT": xT,
            "encT": encT,
            "ebias": eb,
            "wq": wslice_t(Wq, h0),
            "wk": wslice_t(Wk, h0),
            "wv": wslice_t(Wv, h0),
            "wo": np.ascontiguousarray(
                Wo[h0 * D_K:(h0 + HPC) * D_K, :]).astype(np.float16),
            "ident16": ident16,
        })
    return in_maps


def kernel(x, encoding, position_bias, Wq, Wk, Wv, Wo):
    in_maps = _prep_inputs(x, encoding, position_bias,
                           np.asarray(Wq, np.float32),
                           np.asarray(Wk, np.float32),
                           np.asarray(Wv, np.float32),
                           np.asarray(Wo, np.float32))
    nc = _get_nc()
    res = run_bass_kernel_spmd(nc, in_maps, list(range(N_CORES)))
    acc = res.results[0]["out"].astype(np.float32)
    for c in range(1, N_CORES):
        acc = acc + res.results[c]["out"].astype(np.float32)
    return acc
